# revision 22
# baseline (speedup 1.0000x reference)
"""CSPNet GNN message-passing kernel for 8 Trainium2 NeuronCores.

Strategy (graph-data-parallel, 128 crystals per core):
  The edge set is block-diagonal and fully-connected per 20-atom crystal
  (src-major: edge k of graph g is (i=k//20, j=k%20)).  The edge-MLP first
  layer is split by input blocks:
      ein @ W1 = h[src] @ W1a + h[dst] @ W1b + lattice @ W1c + demb @ W1d + b1
  h[src]/h[dst] gathers become zero-stride broadcast access patterns on the
  TensorEngine's moving operand, accumulated in PSUM.  The per-graph term
  (lattice @ W1c + b1) rides in 4 one-hot K-rows appended to the demb
  stationary operand.  The sinusoid table demb is computed once (range-reduced
  Sin on ScalarE) into a DRAM scratch and streamed back each layer.
  Activations are held feature-major ([128 features] x [tokens]); matmul
  operands use float32r (full-rate, ~1e-4 rounding).  Segment mean/graph
  pooling are grouped free-axis reductions on VectorE.
"""
import os
import sys

import numpy as np

try:
    from concourse import bass, mybir, bacc
    import concourse.tile as tile
    from concourse.bass_utils import run_bass_kernel_spmd
except ImportError:  # container staging path
    sys.path.insert(0, "/opt/trn_rl_repo")
    from concourse import bass, mybir, bacc
    import concourse.tile as tile
    from concourse.bass_utils import run_bass_kernel_spmd

F32 = mybir.dt.float32
F32R = mybir.dt.float32r
I32 = mybir.dt.int32
AF = mybir.ActivationFunctionType
ALU = mybir.AluOpType
AX = mybir.AxisListType

G = 1024          # crystals
NA = 20           # atoms per crystal
EPG = NA * NA     # 400 edges per crystal
H = 128           # hidden
LAT = 256         # latent
L = 4             # layers
NF = 10           # freqs
NCORES = 8
GC = G // NCORES          # 128 graphs per core
NN = GC * NA              # 2560 nodes per core
GBLK = 4                  # graphs per pipeline block
NBLK = GC // GBLK         # 32 blocks
NTL = NN // 128           # 20 node tiles of 128
NCH = NN // 512           # 5 node chunks of 512
TWO_PI = float(2 * np.pi)


def _build(sim_mode=False):
    """Build the per-core SPMD program.  sim_mode adapts to CoreSim quirks
    (trunc-cast instead of round, Sigmoid instead of the unimplemented Silu)."""
    silu_fn = AF.Sigmoid if sim_mode else AF.Silu
    dbg = int(os.environ.get("CSP_DEBUG", "0"))
    nc = bacc.Bacc("TRN2", target_bir_lowering=False, debug=False)

    d = {}
    d["t"] = nc.dram_tensor("t_sh", [GC, LAT], F32, kind="ExternalInput")
    d["at"] = nc.dram_tensor("at_sh", [NN], I32, kind="ExternalInput")
    d["frac"] = nc.dram_tensor("frac_sh", [NN, 3], F32, kind="ExternalInput")
    d["lat"] = nc.dram_tensor("lat_sh", [GC, 6], F32, kind="ExternalInput")
    d["nemb"] = nc.dram_tensor("node_emb", [100, H], F32, kind="ExternalInput")
    d["alw"] = nc.dram_tensor("alw", [H + LAT, H], F32, kind="ExternalInput")
    d["alb"] = nc.dram_tensor("alb", [H], F32, kind="ExternalInput")
    d["ew1"] = nc.dram_tensor("ew1", [L, 322, H], F32R, kind="ExternalInput")
    d["eb1"] = nc.dram_tensor("eb1", [L, H], F32, kind="ExternalInput")
    d["ew2"] = nc.dram_tensor("ew2", [L, H, H], F32R, kind="ExternalInput")
    d["eb2"] = nc.dram_tensor("eb2", [L, H], F32, kind="ExternalInput")
    d["nw1"] = nc.dram_tensor("nw1", [L, 2 * H, H], F32R, kind="ExternalInput")
    d["nb1"] = nc.dram_tensor("nb1", [L, H], F32, kind="ExternalInput")
    d["nw2"] = nc.dram_tensor("nw2", [L, H, H], F32R, kind="ExternalInput")
    d["nb2"] = nc.dram_tensor("nb2", [L, H], F32, kind="ExternalInput")
    d["cw"] = nc.dram_tensor("cw", [H, 3], F32R, kind="ExternalInput")
    d["lw"] = nc.dram_tensor("lw", [H, 6], F32R, kind="ExternalInput")
    # host constants
    d["fpmat"] = nc.dram_tensor("fpmat", [4, 32], F32, kind="ExternalInput")
    d["fmmat"] = nc.dram_tensor("fmmat", [4, 32], F32, kind="ExternalInput")
    d["ones"] = nc.dram_tensor("ones_row", [1, NN], F32, kind="ExternalInput")
    d["oh4"] = nc.dram_tensor("onehot4", [4, GBLK * EPG], F32R, kind="ExternalInput")
    d["ident"] = nc.dram_tensor("ident", [H, H], F32, kind="ExternalInput")
    d["clscol"] = nc.dram_tensor("clscol", [100, 1], F32, kind="ExternalInput")
    # outputs (transposed; host untransposes)
    d["latoutT"] = nc.dram_tensor("latoutT", [6, GC], F32, kind="ExternalOutput")
    d["coordT"] = nc.dram_tensor("coordT", [3, NN], F32, kind="ExternalOutput")
    # scratch lives in DRAM pool tiles (dependency-tracked by Tile)
    if dbg:
        d["dbg_h0"] = nc.dram_tensor("dbg_h0", [H, NN], F32, kind="ExternalOutput")
        d["dbg_demb"] = nc.dram_tensor("dbg_demb", [64, EPG], F32,
                                       kind="ExternalOutput")
        d["dbg_agg"] = nc.dram_tensor("dbg_agg", [H, NN], F32,
                                      kind="ExternalOutput")
        d["dbg_h1"] = nc.dram_tensor("dbg_h1", [H, NN], F32, kind="ExternalOutput")
        d["dbg_cb"] = nc.dram_tensor("dbg_cb", [GC, H], F32, kind="ExternalOutput")
        d["dbg_tproj"] = nc.dram_tensor("dbg_tproj", [H, GC], F32,
                                        kind="ExternalOutput")
        d["dbg_ew"] = nc.dram_tensor("dbg_ew", [100, H], F32,
                                     kind="ExternalOutput")
        d["dbg_atbc"] = nc.dram_tensor("dbg_atbc", [100, 2 * 512], F32,
                                       kind="ExternalOutput")

    with tile.TileContext(nc) as tc:
        with (
            tc.tile_pool(name="cst", bufs=1) as cst,
            tc.tile_pool(name="io", bufs=3) as io,
            tc.tile_pool(name="io2", bufs=2) as io2,
            tc.tile_pool(name="io3", bufs=3) as io3,
            tc.tile_pool(name="ps", bufs=2, space="PSUM") as ps,
            tc.tile_pool(name="dpool", bufs=1, space="DRAM") as dpool,
        ):
            demb_scr = dpool.tile([GC, 64, EPG], F32R)
            # ---------- static loads ----------
            ident = cst.tile([H, H], F32)
            nc.sync.dma_start(ident[:], d["ident"][:])
            fpmat = cst.tile([4, 32], F32)
            nc.sync.dma_start(fpmat[:], d["fpmat"][:])
            fmmat = cst.tile([4, 32], F32)
            nc.sync.dma_start(fmmat[:], d["fmmat"][:])
            oh4 = cst.tile([4, GBLK * EPG], F32R)
            nc.sync.dma_start(oh4[:], d["oh4"][:])
            fracT = cst.tile([4, NN], F32)
            nc.sync.dma_start(fracT[0:3, :], d["frac"][:].rearrange("n d -> d n"))
            nc.sync.dma_start(fracT[3:4, :], d["ones"][:])
            clscol = cst.tile([100, 1], F32)
            nc.sync.dma_start(clscol[:], d["clscol"][:])
            t_nat = cst.tile([128, LAT], F32)
            nc.sync.dma_start(t_nat[:], d["t"][:])
            latT1 = cst.tile([7, GC], F32)
            nc.sync.dma_start(latT1[0:6, :], d["lat"][:].rearrange("n d -> d n"))
            nc.sync.dma_start(latT1[6:7, :], d["ones"][:, 0:GC])
            albc = cst.tile([128, 1], F32)
            nc.sync.dma_start(albc[:], d["alb"][:].rearrange("(p o) -> p o", o=1))
            zcol = cst.tile([128, 1], F32)
            nc.vector.memset(zcol[:], 0.0)
            halfpi = cst.tile([128, 1], F32)
            nc.vector.memset(halfpi[:], float(np.pi / 2))
            negpi = cst.tile([128, 1], F32)
            nc.vector.memset(negpi[:], float(-np.pi))

            # Batched weight loads (one DMA per tensor family, issued on the
            # gpsimd queue so the sync queue stays free for data DMAs).
            wab_all = cst.tile([H, L, 2, H], F32R)
            nc.gpsimd.dma_start(
                wab_all[:, :, 0, :],
                d["ew1"][:, 0:H, :].rearrange("l k m -> k l m"))
            nc.gpsimd.dma_start(
                wab_all[:, :, 1, :],
                d["ew1"][:, H:2 * H, :].rearrange("l k m -> k l m"))
            ew2_all = cst.tile([H, L, H], F32R)
            nc.gpsimd.dma_start(ew2_all[:],
                                d["ew2"][:].rearrange("l k m -> k l m"))
            nw1_all = cst.tile([H, L, 2, H], F32R)
            nc.gpsimd.dma_start(
                nw1_all[:, :, 0, :],
                d["nw1"][:, 0:H, :].rearrange("l k m -> k l m"))
            nc.gpsimd.dma_start(
                nw1_all[:, :, 1, :],
                d["nw1"][:, H:2 * H, :].rearrange("l k m -> k l m"))
            nw2_all = cst.tile([H, L, H], F32R)
            nc.gpsimd.dma_start(nw2_all[:],
                                d["nw2"][:].rearrange("l k m -> k l m"))
            w1cb_all = cst.tile([7, L, H], F32)
            nc.gpsimd.dma_start(
                w1cb_all[0:6, :, :],
                d["ew1"][:, 256:262, :].bitcast(F32).rearrange("l k m -> k l m"))
            nc.gpsimd.dma_start(w1cb_all[6:7, :, :], d["eb1"][:].unsqueeze(0))
            eb2c_all = cst.tile([H, L], F32)
            nc.gpsimd.dma_start(eb2c_all[:], d["eb2"][:].rearrange("l p -> p l"))
            nb1c_all = cst.tile([H, L], F32)
            nc.gpsimd.dma_start(nb1c_all[:], d["nb1"][:].rearrange("l p -> p l"))
            nb2c_all = cst.tile([H, L], F32)
            nc.gpsimd.dma_start(nb2c_all[:], d["nb2"][:].rearrange("l p -> p l"))
            alw_all = cst.tile([H, 3, H], F32)
            nc.gpsimd.dma_start(
                alw_all[:], d["alw"][:].rearrange("(c k) m -> k c m", c=3))

            w1a = [wab_all[:, l, 0, :] for l in range(L)]
            w1b = [wab_all[:, l, 1, :] for l in range(L)]
            w2 = [ew2_all[:, l, :] for l in range(L)]
            nw1h = [nw1_all[:, l, 0, :] for l in range(L)]
            nw2t = [nw2_all[:, l, :] for l in range(L)]
            w1cb = [w1cb_all[:, l, :] for l in range(L)]
            eb2c = [eb2c_all[:, l:l + 1] for l in range(L)]
            nb1c = [nb1c_all[:, l:l + 1] for l in range(L)]
            nb2c = [nb2c_all[:, l:l + 1] for l in range(L)]
            nw1a = []
            for l in range(L):
                na_s = cst.tile([H, H], F32R, tag=f"nw1a{l}")
                nc.vector.tensor_scalar_mul(na_s[:], nw1_all[:, l, 1, :],
                                            1.0 / NA)
                nw1a.append(na_s)
            dlhs = []
            for l in range(L):
                pair = []
                for p in range(2):
                    dt_ = cst.tile([64, H], F32R, tag=f"dlhs{l}_{p}")
                    nc.gpsimd.dma_start(dt_[0:60, :], d["ew1"][l, 262:322, :])
                    pair.append(dt_)
                dlhs.append(pair)
            cwS = cst.tile([H, 3], F32R)
            nc.sync.dma_start(cwS[:], d["cw"][:])
            lw_raw = io.tile([H, 6], F32R, tag="lw_raw")
            nc.sync.dma_start(lw_raw[:], d["lw"][:])
            lwS = cst.tile([H, 6], F32R)
            nc.vector.tensor_scalar_mul(lwS[:], lw_raw[:], 1.0 / NA)

            # ---------- EW = node_emb @ Wa  (for the one-hot gather) ----------
            d_ne = cst.tile([100, H], F32)
            nc.sync.dma_start(d_ne[:], d["nemb"][:])
            netp = ps.tile([H, 100], F32, tag="main")
            nc.tensor.transpose(netp[:], d_ne[:], ident[0:100, 0:100])
            nembT = cst.tile([H, 100], F32)
            nc.vector.tensor_copy(nembT[:], netp[:])
            ewp = ps.tile([100, H], F32, tag="main")
            nc.tensor.matmul(ewp[:], nembT[:], alw_all[:, 0, :],
                             start=True, stop=True)
            ew_sb = cst.tile([100, H], F32R)
            nc.vector.tensor_copy(ew_sb[:], ewp[:])

            # ---------- tproj = t @ Wt + alb  (feature-major) ----------
            alw1 = alw_all[:, 1, :]
            alw2 = alw_all[:, 2, :]
            tproj = cst.tile([H, GC], F32)
            tpp = ps.tile([H, GC], F32, tag="main")
            for ci in range(2):
                ttp = ps.tile([H, H], F32, tag="main")
                nc.tensor.transpose(ttp[:], t_nat[:, ci * H:(ci + 1) * H], ident[:])
                tT = io.tile([H, GC], F32, tag="tT")
                nc.vector.tensor_copy(tT[:], ttp[:])
                nc.tensor.matmul(tpp[:], alw1[:] if ci == 0 else alw2[:], tT[:],
                                 start=(ci == 0), stop=(ci == 1))
            nc.vector.tensor_scalar(tproj[:], tpp[:], albc[:, :1], None, op0=ALU.add)

            # ---------- h0 = EW[atom_types] + tproj[graph]  ----------
            # one-hot matmul gather: onehotT[c, n] = (atom_types[n] == c)
            # hbuf0 := tproj[graph(n)], then += EW @ onehot per chunk
            hbufA = cst.tile([H, NN], F32R, tag="hbufA")
            hbufB = cst.tile([H, NN], F32R, tag="hbufB")
            hbuf = [hbufA, hbufB]
            nc.vector.tensor_copy(
                hbuf[0][:].rearrange("p (g r) -> p g r", r=NA),
                tproj[:].unsqueeze(2).broadcast_to([H, GC, NA]))
            for c_ in range(NCH):
                csl = slice(c_ * 512, (c_ + 1) * 512)
                at_bc = io.tile([100, 512], I32, tag="at_bc")
                nc.sync.dma_start(
                    at_bc[:],
                    d["at"][csl].unsqueeze(0).broadcast_to([100, 512]))
                if dbg and c_ < 2:
                    nc.sync.dma_start(d["dbg_atbc"][:, c_ * 512:(c_ + 1) * 512]
                                      .bitcast(I32), at_bc[:])
                oht = io.tile([100, 512], F32R, tag="oht")
                nc.vector.tensor_scalar(oht[:], at_bc[:], clscol[:, :1],
                                        None, op0=ALU.is_equal)
                h0p = ps.tile([H, 512], F32, tag="main")
                nc.tensor.matmul(h0p[:], ew_sb[:], oht[:], start=True, stop=True)
                nc.vector.tensor_tensor(hbuf[0][:, csl], hbuf[0][:, csl],
                                        h0p[:], op=ALU.add)

            if dbg:
                nc.sync.dma_start(d["dbg_h0"][:], hbuf[0][:].bitcast(F32))
                nc.sync.dma_start(d["dbg_tproj"][:], tproj[:])
                nc.sync.dma_start(d["dbg_ew"][:], ew_sb[:].bitcast(F32))

            # ---------- demb scratch (sin/cos table + onehot rows) ----------
            for grp in range(NBLK):
                epm = ps.tile([128, EPG], F32, tag="main")
                for q in range(GBLK):
                    g = grp * GBLK + q
                    sl = epm[32 * q:32 * q + 32, :].rearrange(
                        "p (i j) -> p i j", i=NA)
                    fg = fracT[:, g * NA:(g + 1) * NA]
                    nc.tensor.matmul(sl, fpmat[:],
                                     fg.unsqueeze(1).broadcast_to([4, NA, NA]),
                                     start=True, stop=False,
                                     tile_position=(0, 32 * q))
                    nc.tensor.matmul(sl, fmmat[:],
                                     fg.unsqueeze(2).broadcast_to([4, NA, NA]),
                                     start=False, stop=True,
                                     tile_position=(0, 32 * q))
                xi = io2.tile([128, EPG], I32, tag="xi")
                nc.vector.tensor_copy(xi[:], epm[:])
                z = io2.tile([128, EPG], F32, tag="z")
                nc.vector.tensor_tensor(z[:], epm[:], xi[:], op=ALU.subtract)
                a = io2.tile([128, EPG], F32, tag="a")
                stg = io2.tile([128, 2, EPG], F32R, tag="stg")
                if sim_mode:
                    # trunc-cast: z = frac(x+16.5) in [0,1)
                    z2 = io2.tile([128, EPG], F32, tag="z2")
                    nc.vector.tensor_scalar(z2[:], z[:], 0.5, None,
                                            op0=ALU.subtract)
                    nc.vector.tensor_scalar(
                        a[:].bitcast(mybir.dt.uint32),
                        z2[:].bitcast(mybir.dt.uint32),
                        0x7FFFFFFF, None, op0=ALU.bitwise_and)
                    nc.scalar.activation(stg[:, 0, :], z[:], AF.Sin,
                                         bias=negpi[:, :1], scale=TWO_PI)
                else:
                    # round-cast: z in [-.5,.5]
                    nc.vector.tensor_scalar(
                        a[:].bitcast(mybir.dt.uint32),
                        z[:].bitcast(mybir.dt.uint32),
                        0x7FFFFFFF, None, op0=ALU.bitwise_and)
                    nc.scalar.activation(stg[:, 0, :], z[:], AF.Sin,
                                         bias=zcol[:, :1], scale=TWO_PI)
                nc.scalar.activation(stg[:, 1, :], a[:], AF.Sin,
                                     bias=halfpi[:, :1], scale=-TWO_PI)
                for q in range(GBLK):
                    g = grp * GBLK + q
                    nc.sync.dma_start(
                        demb_scr[g, 0:60, :].rearrange("(c p) n -> p c n", c=2),
                        stg[32 * q:32 * q + 30, :, :])
                    nc.sync.dma_start(demb_scr[g, 60:64, :],
                                      oh4[:, q * EPG:(q + 1) * EPG])

            # ---------- per-layer Cb = lattices @ W1c + b1  (graph-major) ----
            cbnat = []
            for l in range(L):
                cbp = ps.tile([GC, H], F32, tag="main")
                nc.tensor.matmul(cbp[:], latT1[:], w1cb[l][:],
                                 start=True, stop=True)
                cb = cst.tile([GC, H], F32R, tag=f"cbnat{l}")
                nc.vector.tensor_copy(cb[:], cbp[:])
                cbnat.append(cb)

            if dbg:
                dbg_demb_sb = cst.tile([64, EPG], F32)
                nc.sync.dma_start(dbg_demb_sb[:],
                                  demb_scr[0].bitcast(F32))
                nc.sync.dma_start(d["dbg_demb"][:], dbg_demb_sb[:])
                nc.sync.dma_start(d["dbg_cb"][:], cbnat[0][:].bitcast(F32))

            # fence: demb_scr writes must land before layer DMA reads
            tc.strict_bb_all_engine_barrier()

            # ---------- main message-passing layers ----------
            # Software-pipelined: block k's W2/ACT2/reduce are emitted after
            # block k+1's A/B/D matmuls so the PE FIFO never stalls waiting
            # for ACT1 (keeps the HAM clock-gate warm).
            for l in range(L):
                hin = hbuf[l % 2]
                hout = hbuf[(l + 1) % 2]
                aggT = io2.tile([H, NN], F32R, tag="aggT")

                def edge_tail(st, l=l, aggT=aggT):
                    blk_, eb_, xT_ = st
                    for q in range(GBLK):
                        nc.tensor.matmul(eb_[:, q, 0:EPG], w2[l][:],
                                         xT_[:, q, :], start=True, stop=True)
                    nc.scalar.activation(eb_[:, :, 0:EPG], eb_[:, :, 0:EPG],
                                         silu_fn, bias=eb2c[l][:, :1])
                    with nc.allow_low_precision(reason="f32r agg"):
                        nc.vector.tensor_reduce(
                            aggT[:, blk_ * GBLK * NA:(blk_ + 1) * GBLK * NA],
                            eb_[:, :, 0:EPG].rearrange("p g (i j) -> p g i j",
                                                       i=NA),
                            axis=AX.X, op=ALU.add)

                pend = None
                for blk in range(NBLK):
                    g0 = blk * GBLK
                    dmb = io3.tile([64, GBLK, EPG], F32R, tag="dmb")
                    nc.sync.dma_start(
                        dmb[:],
                        demb_scr[g0:g0 + GBLK].rearrange("g p n -> p g n"))
                    dl = dlhs[l][blk % 2]
                    nc.sync.dma_start(dl[60:64, :], cbnat[l][g0:g0 + GBLK, :])
                    eb = ps.tile([128, GBLK, 512], F32, tag="main")
                    gsl = [hin[:, (g0 + q) * NA:(g0 + q + 1) * NA]
                           for q in range(GBLK)]
                    o3 = [eb[:, q, 0:EPG].rearrange("p (i j) -> p i j", i=NA)
                          for q in range(GBLK)]
                    for q in range(GBLK):
                        nc.tensor.matmul(
                            o3[q], w1a[l][:],
                            gsl[q].unsqueeze(2).broadcast_to([H, NA, NA]),
                            start=True, stop=False)
                    for q in range(GBLK):
                        nc.tensor.matmul(
                            o3[q], w1b[l][:],
                            gsl[q].unsqueeze(1).broadcast_to([H, NA, NA]),
                            start=False, stop=False)
                    for q in range(GBLK):
                        nc.tensor.matmul(eb[:, q, 0:EPG], dl[:], dmb[:, q, :],
                                         start=False, stop=True)
                    xT = io3.tile([128, GBLK, EPG], F32R, tag="xT")
                    nc.scalar.activation(xT[:], eb[:, :, 0:EPG], silu_fn,
                                         bias=zcol[:, :1])
                    if pend is not None:
                        edge_tail(pend)
                    pend = (blk, eb, xT)
                edge_tail(pend)
                if dbg and l == 0:
                    nc.sync.dma_start(d["dbg_agg"][:], aggT[:].bitcast(F32))

                def node_tail(st, l=l, hin=hin, hout=hout):
                    csl_, nx_ = st
                    np2 = ps.tile([128, 512], F32, tag="main")
                    nc.tensor.matmul(np2[:], nw2t[l][:], nx_[:],
                                     start=True, stop=True)
                    ny = io.tile([128, 512], F32, tag="ny")
                    nc.scalar.activation(ny[:], np2[:], silu_fn,
                                         bias=nb2c[l][:, :1])
                    nc.vector.tensor_tensor(hout[:, csl_], hin[:, csl_], ny[:],
                                            op=ALU.add)

                pendn = None
                for c in range(NCH):
                    csl = slice(c * 512, (c + 1) * 512)
                    np1 = ps.tile([128, 512], F32, tag="main")
                    nc.tensor.matmul(np1[:], nw1h[l][:], hin[:, csl],
                                     start=True, stop=False)
                    nc.tensor.matmul(np1[:], nw1a[l][:], aggT[:, csl],
                                     start=False, stop=True)
                    nx = io.tile([128, 512], F32R, tag="nx")
                    nc.scalar.activation(nx[:], np1[:], silu_fn,
                                         bias=nb1c[l][:, :1])
                    if pendn is not None:
                        node_tail(pendn)
                    pendn = (csl, nx)
                node_tail(pendn)
                if dbg and l == 0:
                    nc.sync.dma_start(d["dbg_h1"][:], hout[:].bitcast(F32))

            # ---------- outputs ----------
            hfin = hbuf[0]
            gf = io.tile([H, GC], F32R, tag="gf")
            with nc.allow_low_precision(reason="f32r gfeat; fp32 internal"):
                nc.vector.tensor_reduce(
                    gf[:], hfin[:].rearrange("p (g r) -> p g r", r=NA),
                    axis=AX.X, op=ALU.add)
            lop = ps.tile([6, GC], F32, tag="main")
            nc.tensor.matmul(lop[:], lwS[:], gf[:], start=True, stop=True)
            lo_sb = io.tile([6, GC], F32, tag="lo_sb")
            nc.vector.tensor_copy(lo_sb[:], lop[:])
            nc.sync.dma_start(d["latoutT"][:], lo_sb[:])
            co_sb = cst.tile([3, NN], F32)
            for c in range(NCH):
                csl = slice(c * 512, (c + 1) * 512)
                cop = ps.tile([3, 512], F32, tag="main")
                nc.tensor.matmul(cop[:], cwS[:], hfin[:, csl],
                                 start=True, stop=True)
                nc.vector.tensor_copy(co_sb[:, csl], cop[:])
            nc.sync.dma_start(d["coordT"][:], co_sb[:])

    nc.compile()
    return nc


def _host_consts(sim_mode=False):
    fp = np.zeros((4, 32), np.float32)
    for dd in range(3):
        for f in range(NF):
            fp[dd, dd * NF + f] = float(f)
    fm = -fp
    if sim_mode:
        fp = fp.copy()
        fp[3, :30] = 16.5
    oh = np.zeros((4, GBLK * EPG), np.float32)
    for r in range(4):
        oh[r, r * EPG:(r + 1) * EPG] = 1.0
    return {
        "fpmat": fp, "fmmat": fm,
        "ones_row": np.ones((1, NN), np.float32),
        "onehot4": oh,
        "ident": np.eye(H, dtype=np.float32),
        "clscol": np.arange(100, dtype=np.float32).reshape(100, 1),
    }


def _shard(inputs, sim_mode=False):
    consts = _host_consts(sim_mode)
    rep = {
        "node_emb": inputs["node_emb"], "alw": inputs["atom_lat_w"],
        "alb": inputs["atom_lat_b"], "ew1": inputs["edge_w1"],
        "eb1": inputs["edge_b1"], "ew2": inputs["edge_w2"],
        "eb2": inputs["edge_b2"], "nw1": inputs["node_w1"],
        "nb1": inputs["node_b1"], "nw2": inputs["node_w2"],
        "nb2": inputs["node_b2"], "cw": inputs["coord_w"],
        "lw": inputs["lattice_w"],
    }
    rep = {k: np.ascontiguousarray(np.asarray(v, np.float32)) for k, v in rep.items()}
    rep.update(consts)
    in_maps = []
    for c in range(NCORES):
        gs, ge = c * GC, (c + 1) * GC
        ns, ne = c * NN, (c + 1) * NN
        m = dict(rep)
        m["t_sh"] = np.ascontiguousarray(np.asarray(inputs["t"], np.float32)[gs:ge])
        m["at_sh"] = np.ascontiguousarray(np.asarray(inputs["atom_types"], np.int32)[ns:ne])
        m["frac_sh"] = np.ascontiguousarray(np.asarray(inputs["frac_coords"], np.float32)[ns:ne])
        m["lat_sh"] = np.ascontiguousarray(np.asarray(inputs["lattices"], np.float32)[gs:ge])
        in_maps.append(m)
    return in_maps


def _structural_ok(inputs):
    e = np.asarray(inputs["edge_index"])
    if e.shape != (2, G * EPG):
        return False
    base_src = np.repeat(np.arange(NA, dtype=np.int64), NA)
    base_dst = np.tile(np.arange(NA, dtype=np.int64), NA)
    offs = np.repeat(np.arange(G, dtype=np.int64) * NA, EPG)
    if not np.array_equal(e[0], np.tile(base_src, G) + offs):
        return False
    if not np.array_equal(e[1], np.tile(base_dst, G) + offs):
        return False
    n2g = np.asarray(inputs["node2graph"])
    if not np.array_equal(n2g, np.repeat(np.arange(G, dtype=n2g.dtype), NA)):
        return False
    e2g = np.asarray(inputs["edge2graph"])
    if not np.array_equal(e2g, np.repeat(np.arange(G, dtype=e2g.dtype), EPG)):
        return False
    return True


def _numpy_ref(inputs):
    """Exact CPU fallback for non-structural inputs."""
    def silu(x):
        return x / (1.0 + np.exp(-x))
    t = np.asarray(inputs["t"], np.float32)
    at = np.asarray(inputs["atom_types"])
    frac = np.asarray(inputs["frac_coords"], np.float32)
    lats = np.asarray(inputs["lattices"], np.float32)
    n2g = np.asarray(inputs["node2graph"])
    e0, e1 = np.asarray(inputs["edge_index"])
    e2g = np.asarray(inputs["edge2graph"])
    nemb = np.asarray(inputs["node_emb"], np.float32)
    n_nodes = frac.shape[0]
    n_graphs = lats.shape[0]
    fd = np.mod(frac[e1] - frac[e0], 1.0)
    ang = (fd[:, :, None] * (2 * np.pi * np.arange(NF))).reshape(-1, 3 * NF)
    demb = np.concatenate([np.sin(ang), np.cos(ang)], 1).astype(np.float32)
    h = np.concatenate([nemb[at], t[n2g]], 1) @ np.asarray(inputs["atom_lat_w"]) \
        + np.asarray(inputs["atom_lat_b"])
    lat_e = lats[e2g]
    deg = np.maximum(np.bincount(e0, minlength=n_nodes), 1.0).astype(np.float32)
    for i in range(L):
        hi = h
        ein = np.concatenate([h[e0], h[e1], lat_e, demb], 1)
        ef = silu(silu(ein @ inputs["edge_w1"][i] + inputs["edge_b1"][i])
                  @ inputs["edge_w2"][i] + inputs["edge_b2"][i])
        agg = np.zeros((n_nodes, H), np.float32)
        np.add.at(agg, e0, ef)
        agg /= deg[:, None]
        nin = np.concatenate([h, agg], 1)
        h = hi + silu(silu(nin @ inputs["node_w1"][i] + inputs["node_b1"][i])
                      @ inputs["node_w2"][i] + inputs["node_b2"][i])
    cnt = np.maximum(np.bincount(n2g, minlength=n_graphs), 1).astype(np.float32)
    gf = np.zeros((n_graphs, H), np.float32)
    np.add.at(gf, n2g, h)
    gf /= cnt[:, None]
    return (np.asarray(gf @ inputs["lattice_w"], np.float32),
            np.asarray(h @ inputs["coord_w"], np.float32))


_NC_CACHE = {}


def _get_nc(sim_mode=False):
    if sim_mode not in _NC_CACHE:
        _NC_CACHE[sim_mode] = _build(sim_mode)
    return _NC_CACHE[sim_mode]


def run_device(inputs, trace=False):
    nc = _get_nc(False)
    in_maps = _shard(inputs, False)
    res = run_bass_kernel_spmd(nc, in_maps, core_ids=list(range(NCORES)),
                               trace=trace)
    lat = np.concatenate([r["latoutT"].T for r in res.results], 0)
    coord = np.concatenate([r["coordT"].T for r in res.results], 0)
    return (np.ascontiguousarray(lat), np.ascontiguousarray(coord)), res


def kernel(**inputs):
    if not _structural_ok(inputs):
        return _numpy_ref(inputs)
    (lat, coord), _ = run_device(inputs)
    return lat, coord


# revision 23
# speedup vs baseline: 1.2069x; 1.2069x over previous
"""CSPNet GNN message-passing kernel for 8 Trainium2 NeuronCores.

Strategy (graph-data-parallel, 128 crystals per core):
  The edge set is block-diagonal and fully-connected per 20-atom crystal
  (src-major: edge k of graph g is (i=k//20, j=k%20)).  The edge-MLP first
  layer is split by input blocks:
      ein @ W1 = h[src] @ W1a + h[dst] @ W1b + lattice @ W1c + demb @ W1d + b1
  h[src]/h[dst] gathers become zero-stride broadcast access patterns on the
  TensorEngine's moving operand, accumulated in PSUM.  The per-graph term
  (lattice @ W1c + b1) rides in 4 one-hot K-rows appended to the demb
  stationary operand.  The sinusoid table demb is computed once (range-reduced
  Sin on ScalarE) into a DRAM scratch and streamed back each layer.
  Activations are held feature-major ([128 features] x [tokens]); matmul
  operands use float32r (full-rate, ~1e-4 rounding).  Segment mean/graph
  pooling are grouped free-axis reductions on VectorE.
"""
import os
import sys

import numpy as np

try:
    from concourse import bass, mybir, bacc
    import concourse.tile as tile
    from concourse.bass_utils import run_bass_kernel_spmd
except ImportError:  # container staging path
    sys.path.insert(0, "/opt/trn_rl_repo")
    from concourse import bass, mybir, bacc
    import concourse.tile as tile
    from concourse.bass_utils import run_bass_kernel_spmd

F32 = mybir.dt.float32
F32R = mybir.dt.float32r
I32 = mybir.dt.int32
AF = mybir.ActivationFunctionType
ALU = mybir.AluOpType
AX = mybir.AxisListType

G = 1024          # crystals
NA = 20           # atoms per crystal
EPG = NA * NA     # 400 edges per crystal
H = 128           # hidden
LAT = 256         # latent
L = 4             # layers
NF = 10           # freqs
NCORES = 8
GC = G // NCORES          # 128 graphs per core
NN = GC * NA              # 2560 nodes per core
GBLK = 4                  # graphs per pipeline block
NBLK = GC // GBLK         # 32 blocks
NTL = NN // 128           # 20 node tiles of 128
NCH = NN // 512           # 5 node chunks of 512
TWO_PI = float(2 * np.pi)


def _build(sim_mode=False):
    """Build the per-core SPMD program.  sim_mode adapts to CoreSim quirks
    (trunc-cast instead of round, Sigmoid instead of the unimplemented Silu)."""
    silu_fn = AF.Sigmoid if sim_mode else AF.Silu
    dbg = int(os.environ.get("CSP_DEBUG", "0"))
    nc = bacc.Bacc("TRN2", target_bir_lowering=False, debug=False)

    d = {}
    d["t"] = nc.dram_tensor("t_sh", [GC, LAT], F32, kind="ExternalInput")
    d["at"] = nc.dram_tensor("at_sh", [NN], I32, kind="ExternalInput")
    d["frac"] = nc.dram_tensor("frac_sh", [NN, 3], F32, kind="ExternalInput")
    d["lat"] = nc.dram_tensor("lat_sh", [GC, 6], F32, kind="ExternalInput")
    d["nemb"] = nc.dram_tensor("node_emb", [100, H], F32, kind="ExternalInput")
    d["alw"] = nc.dram_tensor("alw", [H + LAT, H], F32, kind="ExternalInput")
    d["alb"] = nc.dram_tensor("alb", [H], F32, kind="ExternalInput")
    d["ew1"] = nc.dram_tensor("ew1", [L, 322, H], F32R, kind="ExternalInput")
    d["eb1"] = nc.dram_tensor("eb1", [L, H], F32, kind="ExternalInput")
    d["ew2"] = nc.dram_tensor("ew2", [L, H, H], F32R, kind="ExternalInput")
    d["eb2"] = nc.dram_tensor("eb2", [L, H], F32, kind="ExternalInput")
    d["nw1"] = nc.dram_tensor("nw1", [L, 2 * H, H], F32R, kind="ExternalInput")
    d["nb1"] = nc.dram_tensor("nb1", [L, H], F32, kind="ExternalInput")
    d["nw2"] = nc.dram_tensor("nw2", [L, H, H], F32R, kind="ExternalInput")
    d["nb2"] = nc.dram_tensor("nb2", [L, H], F32, kind="ExternalInput")
    d["cw"] = nc.dram_tensor("cw", [H, 3], F32R, kind="ExternalInput")
    d["lw"] = nc.dram_tensor("lw", [H, 6], F32R, kind="ExternalInput")
    # host constants
    d["fpmat"] = nc.dram_tensor("fpmat", [4, 32], F32, kind="ExternalInput")
    d["fmmat"] = nc.dram_tensor("fmmat", [4, 32], F32, kind="ExternalInput")
    d["ones"] = nc.dram_tensor("ones_row", [1, NN], F32, kind="ExternalInput")
    d["oh4"] = nc.dram_tensor("onehot4", [4, GBLK * EPG], F32R, kind="ExternalInput")
    d["ident"] = nc.dram_tensor("ident", [H, H], F32, kind="ExternalInput")
    d["clscol"] = nc.dram_tensor("clscol", [100, 1], F32, kind="ExternalInput")
    # outputs (transposed; host untransposes)
    d["latoutT"] = nc.dram_tensor("latoutT", [6, GC], F32, kind="ExternalOutput")
    d["coordT"] = nc.dram_tensor("coordT", [3, NN], F32, kind="ExternalOutput")
    # scratch lives in DRAM pool tiles (dependency-tracked by Tile)
    if dbg:
        d["dbg_h0"] = nc.dram_tensor("dbg_h0", [H, NN], F32, kind="ExternalOutput")
        d["dbg_demb"] = nc.dram_tensor("dbg_demb", [64, EPG], F32,
                                       kind="ExternalOutput")
        d["dbg_agg"] = nc.dram_tensor("dbg_agg", [H, NN], F32,
                                      kind="ExternalOutput")
        d["dbg_h1"] = nc.dram_tensor("dbg_h1", [H, NN], F32, kind="ExternalOutput")
        d["dbg_cb"] = nc.dram_tensor("dbg_cb", [GC, H], F32, kind="ExternalOutput")
        d["dbg_tproj"] = nc.dram_tensor("dbg_tproj", [H, GC], F32,
                                        kind="ExternalOutput")
        d["dbg_ew"] = nc.dram_tensor("dbg_ew", [100, H], F32,
                                     kind="ExternalOutput")
        d["dbg_atbc"] = nc.dram_tensor("dbg_atbc", [100, 2 * 512], F32,
                                       kind="ExternalOutput")

    with tile.TileContext(nc) as tc:
        with (
            tc.tile_pool(name="cst", bufs=1) as cst,
            tc.tile_pool(name="io", bufs=3) as io,
            tc.tile_pool(name="io2", bufs=2) as io2,
            tc.tile_pool(name="io3", bufs=3) as io3,
            tc.tile_pool(name="ps", bufs=2, space="PSUM") as ps,
            tc.tile_pool(name="dpool", bufs=1, space="DRAM") as dpool,
        ):
            demb_scr = dpool.tile([GC, 64, EPG], F32R)
            # ---------- static loads ----------
            ident = cst.tile([H, H], F32)
            nc.sync.dma_start(ident[:], d["ident"][:])
            fpmat = cst.tile([4, 32], F32)
            nc.sync.dma_start(fpmat[:], d["fpmat"][:])
            fmmat = cst.tile([4, 32], F32)
            nc.sync.dma_start(fmmat[:], d["fmmat"][:])
            oh4 = cst.tile([4, GBLK * EPG], F32R)
            nc.sync.dma_start(oh4[:], d["oh4"][:])
            fracT = cst.tile([4, NN], F32)
            nc.sync.dma_start(fracT[0:3, :], d["frac"][:].rearrange("n d -> d n"))
            nc.sync.dma_start(fracT[3:4, :], d["ones"][:])
            clscol = cst.tile([100, 1], F32)
            nc.sync.dma_start(clscol[:], d["clscol"][:])
            t_nat = cst.tile([128, LAT], F32)
            nc.sync.dma_start(t_nat[:], d["t"][:])
            latT1 = cst.tile([7, GC], F32)
            nc.sync.dma_start(latT1[0:6, :], d["lat"][:].rearrange("n d -> d n"))
            nc.sync.dma_start(latT1[6:7, :], d["ones"][:, 0:GC])
            albc = cst.tile([128, 1], F32)
            nc.sync.dma_start(albc[:], d["alb"][:].rearrange("(p o) -> p o", o=1))
            zcol = cst.tile([128, 1], F32)
            nc.vector.memset(zcol[:], 0.0)
            halfpi = cst.tile([128, 1], F32)
            nc.vector.memset(halfpi[:], float(np.pi / 2))
            negpi = cst.tile([128, 1], F32)
            nc.vector.memset(negpi[:], float(-np.pi))

            # Batched weight loads (one DMA per tensor family, issued on the
            # gpsimd queue so the sync queue stays free for data DMAs).
            wab_all = cst.tile([H, L, 2, H], F32R)
            nc.gpsimd.dma_start(
                wab_all[:, :, 0, :],
                d["ew1"][:, 0:H, :].rearrange("l k m -> k l m"))
            nc.gpsimd.dma_start(
                wab_all[:, :, 1, :],
                d["ew1"][:, H:2 * H, :].rearrange("l k m -> k l m"))
            ew2_all = cst.tile([H, L, H], F32R)
            nc.gpsimd.dma_start(ew2_all[:],
                                d["ew2"][:].rearrange("l k m -> k l m"))
            nw1_all = cst.tile([H, L, 2, H], F32R)
            nc.gpsimd.dma_start(
                nw1_all[:, :, 0, :],
                d["nw1"][:, 0:H, :].rearrange("l k m -> k l m"))
            nc.gpsimd.dma_start(
                nw1_all[:, :, 1, :],
                d["nw1"][:, H:2 * H, :].rearrange("l k m -> k l m"))
            nw2_all = cst.tile([H, L, H], F32R)
            nc.gpsimd.dma_start(nw2_all[:],
                                d["nw2"][:].rearrange("l k m -> k l m"))
            w1cb_all = cst.tile([7, L, H], F32)
            nc.gpsimd.dma_start(
                w1cb_all[0:6, :, :],
                d["ew1"][:, 256:262, :].bitcast(F32).rearrange("l k m -> k l m"))
            nc.gpsimd.dma_start(w1cb_all[6:7, :, :], d["eb1"][:].unsqueeze(0))
            eb2c_all = cst.tile([H, L], F32)
            nc.gpsimd.dma_start(eb2c_all[:], d["eb2"][:].rearrange("l p -> p l"))
            nb1c_all = cst.tile([H, L], F32)
            nc.gpsimd.dma_start(nb1c_all[:], d["nb1"][:].rearrange("l p -> p l"))
            nb2c_all = cst.tile([H, L], F32)
            nc.gpsimd.dma_start(nb2c_all[:], d["nb2"][:].rearrange("l p -> p l"))
            alw_all = cst.tile([H, 3, H], F32)
            nc.gpsimd.dma_start(
                alw_all[:], d["alw"][:].rearrange("(c k) m -> k c m", c=3))

            w1a = [wab_all[:, l, 0, :] for l in range(L)]
            w1b = [wab_all[:, l, 1, :] for l in range(L)]
            w2 = [ew2_all[:, l, :] for l in range(L)]
            nw1h = [nw1_all[:, l, 0, :] for l in range(L)]
            nw2t = [nw2_all[:, l, :] for l in range(L)]
            w1cb = [w1cb_all[:, l, :] for l in range(L)]
            eb2c = [eb2c_all[:, l:l + 1] for l in range(L)]
            nb1c = [nb1c_all[:, l:l + 1] for l in range(L)]
            nb2c = [nb2c_all[:, l:l + 1] for l in range(L)]
            nw1a = []
            for l in range(L):
                na_s = cst.tile([H, H], F32R, tag=f"nw1a{l}")
                nc.vector.tensor_scalar_mul(na_s[:], nw1_all[:, l, 1, :],
                                            1.0 / NA)
                nw1a.append(na_s)
            dlhs = []
            for l in range(L):
                pair = []
                for p in range(2):
                    dt_ = cst.tile([64, H], F32R, tag=f"dlhs{l}_{p}")
                    nc.gpsimd.dma_start(dt_[0:60, :], d["ew1"][l, 262:322, :])
                    pair.append(dt_)
                dlhs.append(pair)
            cwS = cst.tile([H, 3], F32R)
            nc.sync.dma_start(cwS[:], d["cw"][:])
            lw_raw = io.tile([H, 6], F32R, tag="lw_raw")
            nc.sync.dma_start(lw_raw[:], d["lw"][:])
            lwS = cst.tile([H, 6], F32R)
            nc.vector.tensor_scalar_mul(lwS[:], lw_raw[:], 1.0 / NA)

            # ---------- EW = node_emb @ Wa  (for the one-hot gather) ----------
            d_ne = cst.tile([100, H], F32)
            nc.sync.dma_start(d_ne[:], d["nemb"][:])
            netp = ps.tile([H, 100], F32, tag="main")
            nc.tensor.transpose(netp[:], d_ne[:], ident[0:100, 0:100])
            nembT = cst.tile([H, 100], F32)
            nc.vector.tensor_copy(nembT[:], netp[:])
            ewp = ps.tile([100, H], F32, tag="main")
            nc.tensor.matmul(ewp[:], nembT[:], alw_all[:, 0, :],
                             start=True, stop=True)
            ew_sb = cst.tile([100, H], F32R)
            nc.vector.tensor_copy(ew_sb[:], ewp[:])

            # ---------- tproj = t @ Wt + alb  (feature-major) ----------
            alw1 = alw_all[:, 1, :]
            alw2 = alw_all[:, 2, :]
            tproj = cst.tile([H, GC], F32)
            tpp = ps.tile([H, GC], F32, tag="main")
            for ci in range(2):
                ttp = ps.tile([H, H], F32, tag="main")
                nc.tensor.transpose(ttp[:], t_nat[:, ci * H:(ci + 1) * H], ident[:])
                tT = io.tile([H, GC], F32, tag="tT")
                nc.vector.tensor_copy(tT[:], ttp[:])
                nc.tensor.matmul(tpp[:], alw1[:] if ci == 0 else alw2[:], tT[:],
                                 start=(ci == 0), stop=(ci == 1))
            nc.vector.tensor_scalar(tproj[:], tpp[:], albc[:, :1], None, op0=ALU.add)

            # ---------- h0 = EW[atom_types] + tproj[graph]  ----------
            # one-hot matmul gather: onehotT[c, n] = (atom_types[n] == c)
            # hbuf0 := tproj[graph(n)], then += EW @ onehot per chunk
            hbufA = cst.tile([H, NN], F32R, tag="hbufA")
            hbufB = cst.tile([H, NN], F32R, tag="hbufB")
            hbuf = [hbufA, hbufB]
            nc.vector.tensor_copy(
                hbuf[0][:].rearrange("p (g r) -> p g r", r=NA),
                tproj[:].unsqueeze(2).broadcast_to([H, GC, NA]))
            for c_ in range(NCH):
                csl = slice(c_ * 512, (c_ + 1) * 512)
                at_bc = io.tile([100, 512], I32, tag="at_bc")
                nc.sync.dma_start(
                    at_bc[:],
                    d["at"][csl].unsqueeze(0).broadcast_to([100, 512]))
                if dbg and c_ < 2:
                    nc.sync.dma_start(d["dbg_atbc"][:, c_ * 512:(c_ + 1) * 512]
                                      .bitcast(I32), at_bc[:])
                oht = io.tile([100, 512], F32R, tag="oht")
                nc.vector.tensor_scalar(oht[:], at_bc[:], clscol[:, :1],
                                        None, op0=ALU.is_equal)
                h0p = ps.tile([H, 512], F32, tag="main")
                nc.tensor.matmul(h0p[:], ew_sb[:], oht[:], start=True, stop=True)
                nc.vector.tensor_tensor(hbuf[0][:, csl], hbuf[0][:, csl],
                                        h0p[:], op=ALU.add)

            if dbg:
                nc.sync.dma_start(d["dbg_h0"][:], hbuf[0][:].bitcast(F32))
                nc.sync.dma_start(d["dbg_tproj"][:], tproj[:])
                nc.sync.dma_start(d["dbg_ew"][:], ew_sb[:].bitcast(F32))

            # ---------- demb scratch (sin/cos table + onehot rows) ----------
            for grp in range(NBLK):
                epm = ps.tile([128, EPG], F32, tag="main")
                for q in range(GBLK):
                    g = grp * GBLK + q
                    sl = epm[32 * q:32 * q + 32, :].rearrange(
                        "p (i j) -> p i j", i=NA)
                    fg = fracT[:, g * NA:(g + 1) * NA]
                    nc.tensor.matmul(sl, fpmat[:],
                                     fg.unsqueeze(1).broadcast_to([4, NA, NA]),
                                     start=True, stop=False,
                                     tile_position=(0, 32 * q))
                    nc.tensor.matmul(sl, fmmat[:],
                                     fg.unsqueeze(2).broadcast_to([4, NA, NA]),
                                     start=False, stop=True,
                                     tile_position=(0, 32 * q))
                xi = io2.tile([128, EPG], I32, tag="xi")
                nc.vector.tensor_copy(xi[:], epm[:])
                z = io2.tile([128, EPG], F32, tag="z")
                nc.vector.tensor_tensor(z[:], epm[:], xi[:], op=ALU.subtract)
                a = io2.tile([128, EPG], F32, tag="a")
                stg = io2.tile([128, 2, EPG], F32R, tag="stg")
                if sim_mode:
                    # trunc-cast: z = frac(x+16.5) in [0,1)
                    z2 = io2.tile([128, EPG], F32, tag="z2")
                    nc.vector.tensor_scalar(z2[:], z[:], 0.5, None,
                                            op0=ALU.subtract)
                    nc.vector.tensor_scalar(
                        a[:].bitcast(mybir.dt.uint32),
                        z2[:].bitcast(mybir.dt.uint32),
                        0x7FFFFFFF, None, op0=ALU.bitwise_and)
                    nc.scalar.activation(stg[:, 0, :], z[:], AF.Sin,
                                         bias=negpi[:, :1], scale=TWO_PI)
                else:
                    # round-cast: z in [-.5,.5]
                    nc.vector.tensor_scalar(
                        a[:].bitcast(mybir.dt.uint32),
                        z[:].bitcast(mybir.dt.uint32),
                        0x7FFFFFFF, None, op0=ALU.bitwise_and)
                    nc.scalar.activation(stg[:, 0, :], z[:], AF.Sin,
                                         bias=zcol[:, :1], scale=TWO_PI)
                nc.scalar.activation(stg[:, 1, :], a[:], AF.Sin,
                                     bias=halfpi[:, :1], scale=-TWO_PI)
                for q in range(GBLK):
                    g = grp * GBLK + q
                    nc.sync.dma_start(
                        demb_scr[g, 0:60, :].rearrange("(c p) n -> p c n", c=2),
                        stg[32 * q:32 * q + 30, :, :])
                    nc.sync.dma_start(demb_scr[g, 60:64, :],
                                      oh4[:, q * EPG:(q + 1) * EPG])

            # ---------- per-layer Cb = lattices @ W1c + b1  (graph-major) ----
            cbnat = []
            for l in range(L):
                cbp = ps.tile([GC, H], F32, tag="main")
                nc.tensor.matmul(cbp[:], latT1[:], w1cb[l][:],
                                 start=True, stop=True)
                cb = cst.tile([GC, H], F32R, tag=f"cbnat{l}")
                nc.vector.tensor_copy(cb[:], cbp[:])
                cbnat.append(cb)

            if dbg:
                dbg_demb_sb = cst.tile([64, EPG], F32)
                nc.sync.dma_start(dbg_demb_sb[:],
                                  demb_scr[0].bitcast(F32))
                nc.sync.dma_start(d["dbg_demb"][:], dbg_demb_sb[:])
                nc.sync.dma_start(d["dbg_cb"][:], cbnat[0][:].bitcast(F32))

            # fence: demb_scr writes must land before layer DMA reads
            tc.strict_bb_all_engine_barrier()

            # ---------- main message-passing layers ----------
            # Software-pipelined: block k's W2/ACT2/reduce are emitted after
            # block k+1's A/B/D matmuls so the PE FIFO never stalls waiting
            # for ACT1 (keeps the HAM clock-gate warm).
            for l in range(L):
                hin = hbuf[l % 2]
                hout = hbuf[(l + 1) % 2]
                aggT = io2.tile([H, NN], F32R, tag="aggT")

                def edge_tail(st, l=l, aggT=aggT):
                    blk_, eb_, xT_ = st
                    for q in range(GBLK):
                        nc.tensor.matmul(eb_[:, q, 0:EPG], w2[l][:],
                                         xT_[:, q, :], start=True, stop=True)
                    ef = io2.tile([128, GBLK, EPG], F32, tag="ef")
                    nc.scalar.activation(ef[:], eb_[:, :, 0:EPG], silu_fn,
                                         bias=eb2c[l][:, :1])
                    with nc.allow_low_precision(reason="f32r agg"):
                        nc.vector.tensor_reduce(
                            aggT[:, blk_ * GBLK * NA:(blk_ + 1) * GBLK * NA],
                            ef[:].rearrange("p g (i j) -> p g i j", i=NA),
                            axis=AX.X, op=ALU.add)

                pend = None
                for blk in range(NBLK):
                    g0 = blk * GBLK
                    dmb = io3.tile([64, GBLK, EPG], F32R, tag="dmb")
                    nc.sync.dma_start(
                        dmb[:],
                        demb_scr[g0:g0 + GBLK].rearrange("g p n -> p g n"))
                    dl = dlhs[l][blk % 2]
                    nc.sync.dma_start(dl[60:64, :], cbnat[l][g0:g0 + GBLK, :])
                    eb = ps.tile([128, GBLK, 512], F32, tag="main")
                    gsl = [hin[:, (g0 + q) * NA:(g0 + q + 1) * NA]
                           for q in range(GBLK)]
                    o3 = [eb[:, q, 0:EPG].rearrange("p (i j) -> p i j", i=NA)
                          for q in range(GBLK)]
                    for q in range(GBLK):
                        nc.tensor.matmul(
                            o3[q], w1a[l][:],
                            gsl[q].unsqueeze(2).broadcast_to([H, NA, NA]),
                            start=True, stop=False)
                    for q in range(GBLK):
                        nc.tensor.matmul(
                            o3[q], w1b[l][:],
                            gsl[q].unsqueeze(1).broadcast_to([H, NA, NA]),
                            start=False, stop=False)
                    for q in range(GBLK):
                        nc.tensor.matmul(eb[:, q, 0:EPG], dl[:], dmb[:, q, :],
                                         start=False, stop=True)
                    xT = io3.tile([128, GBLK, EPG], F32R, tag="xT")
                    nc.scalar.activation(xT[:], eb[:, :, 0:EPG], silu_fn,
                                         bias=zcol[:, :1])
                    if pend is not None:
                        edge_tail(pend)
                    pend = (blk, eb, xT)
                edge_tail(pend)
                if dbg and l == 0:
                    nc.sync.dma_start(d["dbg_agg"][:], aggT[:].bitcast(F32))

                def node_tail(st, l=l, hin=hin, hout=hout):
                    csl_, nx_ = st
                    np2 = ps.tile([128, 512], F32, tag="main")
                    nc.tensor.matmul(np2[:], nw2t[l][:], nx_[:],
                                     start=True, stop=True)
                    ny = io.tile([128, 512], F32, tag="ny")
                    nc.scalar.activation(ny[:], np2[:], silu_fn,
                                         bias=nb2c[l][:, :1])
                    nc.vector.tensor_tensor(hout[:, csl_], hin[:, csl_], ny[:],
                                            op=ALU.add)

                pendn = None
                for c in range(NCH):
                    csl = slice(c * 512, (c + 1) * 512)
                    np1 = ps.tile([128, 512], F32, tag="main")
                    nc.tensor.matmul(np1[:], nw1h[l][:], hin[:, csl],
                                     start=True, stop=False)
                    nc.tensor.matmul(np1[:], nw1a[l][:], aggT[:, csl],
                                     start=False, stop=True)
                    nx = io.tile([128, 512], F32R, tag="nx")
                    nc.scalar.activation(nx[:], np1[:], silu_fn,
                                         bias=nb1c[l][:, :1])
                    if pendn is not None:
                        node_tail(pendn)
                    pendn = (csl, nx)
                node_tail(pendn)
                if dbg and l == 0:
                    nc.sync.dma_start(d["dbg_h1"][:], hout[:].bitcast(F32))

            # ---------- outputs ----------
            hfin = hbuf[0]
            gf = io.tile([H, GC], F32R, tag="gf")
            with nc.allow_low_precision(reason="f32r gfeat; fp32 internal"):
                nc.vector.tensor_reduce(
                    gf[:], hfin[:].rearrange("p (g r) -> p g r", r=NA),
                    axis=AX.X, op=ALU.add)
            lop = ps.tile([6, GC], F32, tag="main")
            nc.tensor.matmul(lop[:], lwS[:], gf[:], start=True, stop=True)
            lo_sb = io.tile([6, GC], F32, tag="lo_sb")
            nc.vector.tensor_copy(lo_sb[:], lop[:])
            nc.sync.dma_start(d["latoutT"][:], lo_sb[:])
            co_sb = cst.tile([3, NN], F32)
            for c in range(NCH):
                csl = slice(c * 512, (c + 1) * 512)
                cop = ps.tile([3, 512], F32, tag="main")
                nc.tensor.matmul(cop[:], cwS[:], hfin[:, csl],
                                 start=True, stop=True)
                nc.vector.tensor_copy(co_sb[:, csl], cop[:])
            nc.sync.dma_start(d["coordT"][:], co_sb[:])

    nc.compile()
    return nc


def _host_consts(sim_mode=False):
    fp = np.zeros((4, 32), np.float32)
    for dd in range(3):
        for f in range(NF):
            fp[dd, dd * NF + f] = float(f)
    fm = -fp
    if sim_mode:
        fp = fp.copy()
        fp[3, :30] = 16.5
    oh = np.zeros((4, GBLK * EPG), np.float32)
    for r in range(4):
        oh[r, r * EPG:(r + 1) * EPG] = 1.0
    return {
        "fpmat": fp, "fmmat": fm,
        "ones_row": np.ones((1, NN), np.float32),
        "onehot4": oh,
        "ident": np.eye(H, dtype=np.float32),
        "clscol": np.arange(100, dtype=np.float32).reshape(100, 1),
    }


def _shard(inputs, sim_mode=False):
    consts = _host_consts(sim_mode)
    rep = {
        "node_emb": inputs["node_emb"], "alw": inputs["atom_lat_w"],
        "alb": inputs["atom_lat_b"], "ew1": inputs["edge_w1"],
        "eb1": inputs["edge_b1"], "ew2": inputs["edge_w2"],
        "eb2": inputs["edge_b2"], "nw1": inputs["node_w1"],
        "nb1": inputs["node_b1"], "nw2": inputs["node_w2"],
        "nb2": inputs["node_b2"], "cw": inputs["coord_w"],
        "lw": inputs["lattice_w"],
    }
    rep = {k: np.ascontiguousarray(np.asarray(v, np.float32)) for k, v in rep.items()}
    rep.update(consts)
    in_maps = []
    for c in range(NCORES):
        gs, ge = c * GC, (c + 1) * GC
        ns, ne = c * NN, (c + 1) * NN
        m = dict(rep)
        m["t_sh"] = np.ascontiguousarray(np.asarray(inputs["t"], np.float32)[gs:ge])
        m["at_sh"] = np.ascontiguousarray(np.asarray(inputs["atom_types"], np.int32)[ns:ne])
        m["frac_sh"] = np.ascontiguousarray(np.asarray(inputs["frac_coords"], np.float32)[ns:ne])
        m["lat_sh"] = np.ascontiguousarray(np.asarray(inputs["lattices"], np.float32)[gs:ge])
        in_maps.append(m)
    return in_maps


def _structural_ok(inputs):
    e = np.asarray(inputs["edge_index"])
    if e.shape != (2, G * EPG):
        return False
    base_src = np.repeat(np.arange(NA, dtype=np.int64), NA)
    base_dst = np.tile(np.arange(NA, dtype=np.int64), NA)
    offs = np.repeat(np.arange(G, dtype=np.int64) * NA, EPG)
    if not np.array_equal(e[0], np.tile(base_src, G) + offs):
        return False
    if not np.array_equal(e[1], np.tile(base_dst, G) + offs):
        return False
    n2g = np.asarray(inputs["node2graph"])
    if not np.array_equal(n2g, np.repeat(np.arange(G, dtype=n2g.dtype), NA)):
        return False
    e2g = np.asarray(inputs["edge2graph"])
    if not np.array_equal(e2g, np.repeat(np.arange(G, dtype=e2g.dtype), EPG)):
        return False
    return True


def _numpy_ref(inputs):
    """Exact CPU fallback for non-structural inputs."""
    def silu(x):
        return x / (1.0 + np.exp(-x))
    t = np.asarray(inputs["t"], np.float32)
    at = np.asarray(inputs["atom_types"])
    frac = np.asarray(inputs["frac_coords"], np.float32)
    lats = np.asarray(inputs["lattices"], np.float32)
    n2g = np.asarray(inputs["node2graph"])
    e0, e1 = np.asarray(inputs["edge_index"])
    e2g = np.asarray(inputs["edge2graph"])
    nemb = np.asarray(inputs["node_emb"], np.float32)
    n_nodes = frac.shape[0]
    n_graphs = lats.shape[0]
    fd = np.mod(frac[e1] - frac[e0], 1.0)
    ang = (fd[:, :, None] * (2 * np.pi * np.arange(NF))).reshape(-1, 3 * NF)
    demb = np.concatenate([np.sin(ang), np.cos(ang)], 1).astype(np.float32)
    h = np.concatenate([nemb[at], t[n2g]], 1) @ np.asarray(inputs["atom_lat_w"]) \
        + np.asarray(inputs["atom_lat_b"])
    lat_e = lats[e2g]
    deg = np.maximum(np.bincount(e0, minlength=n_nodes), 1.0).astype(np.float32)
    for i in range(L):
        hi = h
        ein = np.concatenate([h[e0], h[e1], lat_e, demb], 1)
        ef = silu(silu(ein @ inputs["edge_w1"][i] + inputs["edge_b1"][i])
                  @ inputs["edge_w2"][i] + inputs["edge_b2"][i])
        agg = np.zeros((n_nodes, H), np.float32)
        np.add.at(agg, e0, ef)
        agg /= deg[:, None]
        nin = np.concatenate([h, agg], 1)
        h = hi + silu(silu(nin @ inputs["node_w1"][i] + inputs["node_b1"][i])
                      @ inputs["node_w2"][i] + inputs["node_b2"][i])
    cnt = np.maximum(np.bincount(n2g, minlength=n_graphs), 1).astype(np.float32)
    gf = np.zeros((n_graphs, H), np.float32)
    np.add.at(gf, n2g, h)
    gf /= cnt[:, None]
    return (np.asarray(gf @ inputs["lattice_w"], np.float32),
            np.asarray(h @ inputs["coord_w"], np.float32))


_NC_CACHE = {}


def _get_nc(sim_mode=False):
    if sim_mode not in _NC_CACHE:
        _NC_CACHE[sim_mode] = _build(sim_mode)
    return _NC_CACHE[sim_mode]


def run_device(inputs, trace=False):
    nc = _get_nc(False)
    in_maps = _shard(inputs, False)
    res = run_bass_kernel_spmd(nc, in_maps, core_ids=list(range(NCORES)),
                               trace=trace)
    lat = np.concatenate([r["latoutT"].T for r in res.results], 0)
    coord = np.concatenate([r["coordT"].T for r in res.results], 0)
    return (np.ascontiguousarray(lat), np.ascontiguousarray(coord)), res


def kernel(**inputs):
    if not _structural_ok(inputs):
        return _numpy_ref(inputs)
    (lat, coord), _ = run_device(inputs)
    return lat, coord
